# revision 3
# baseline (speedup 1.0000x reference)
# Trainium2 Bass kernel for nn_AttentionCombiner — collective-free version.
#
# Sharding: 8 cores = 4 batches x 2 q-halves (as baseline), but ALL
# communication is eliminated. The host feeds each core X with its l-blocks
# PERMUTED so that blocks 0..7 are the rows matching the core's own q-range
# (so xt[:, 0:QH] doubles as the q-side operand). Per head:
#   * tiles 0..7 form the symmetric square A = S[my-l, my-q]; ACT accum_out
#     on these tiles gives column-sums of A (== row-sums over my half) in
#     f32, already laid out [q-partition, block] for the fc combine.
#   * tiles 8..15 (other half's rows, no diagonal, all values O(1)) are
#     accumulated on DVE in bf16, partition-reduced in one gpsimd
#     partition_all_reduce, and bounced through DRAM (4KB) into
#     [q-partition, block] layout.
# r = A-part + B-part, recip = 1/r; fc combine = baseline's
# scalar_tensor_tensor path, unchanged.
#
# Schedule: per head, the B tiles run FIRST (slots 0..7, carrying the DVE
# bf16 tile-sum) so the row-sum pipeline (Pool reduce at slot 8, DRAM bounce
# at slot 10) finishes within the same head; the previous head's deferred
# steps (rowsum combine + 8 fc steps) pop one per slot in slots 6..14,
# keeping DVE evenly loaded. ACT (exp stream) is the machine bottleneck at
# ~146us busy / 89% occupancy; PE ~127us, DVE ~89us, Pool ~13us.

import numpy as np
import ml_dtypes

N, S, D_IN, HEADS = 4, 2048, 512, 8
HEAD_DIM = 128          # 2*D_IN // HEADS
DF = 2 * D_IN           # 1024 combined features
QH = S // 2             # 1024 rows per core
NB = S // 128           # 16 l-blocks
HB = NB // 2            # 8 blocks in my half
QB = QH // 128          # 8 q-blocks per core
ISQ = 1.0 / float(np.sqrt(np.float32(HEAD_DIM)))

_CACHED_NC = None


def _build_nc():
    import concourse.mybir as mybir
    import concourse.tile as tile
    from concourse import bacc, bass_isa
    from concourse.bass import ts

    f32 = mybir.dt.float32
    bf16 = mybir.dt.bfloat16
    Exp = mybir.ActivationFunctionType.Exp
    mult = mybir.AluOpType.mult
    add = mybir.AluOpType.add
    radd = bass_isa.ReduceOp.add

    nc = bacc.Bacc("TRN2", target_bir_lowering=False, debug=False, num_devices=8)

    xt = nc.dram_tensor("xt", [DF, S], bf16, kind="ExternalInput")
    x = nc.dram_tensor("x", [S, DF], bf16, kind="ExternalInput")  # same l-perm
    w = nc.dram_tensor("w", [DF, D_IN], bf16, kind="ExternalInput")
    bias = nc.dram_tensor("bias", [128, D_IN], f32, kind="ExternalInput")
    out = nc.dram_tensor("out", [QH, D_IN], f32, kind="ExternalOutput")

    # B tiles (other half's rows) first, then the symmetric-square tiles.
    ORDER = list(range(HB, NB)) + list(range(HB))

    with tile.TileContext(nc) as tc:
        with (
            tc.tile_pool(name="persist", bufs=1) as pers,
            tc.tile_pool(name="spool", bufs=6) as spool,
            tc.tile_pool(name="outp", bufs=3) as outp,
            tc.tile_pool(name="tsb", bufs=2) as tsb,
            tc.tile_pool(name="rpool", bufs=2) as rpool,
            tc.tile_pool(name="psE", bufs=2, space="PSUM") as psE,
            tc.tile_pool(name="psO", bufs=1, space="PSUM") as psO,
            tc.tile_pool(name="psFC", bufs=2, space="PSUM") as psFC,
            tc.tile_pool(name="dram", bufs=1, space="DRAM") as dram,
        ):
            # ---- persistent SBUF data ----
            xt_sb = pers.tile([128, HEADS, S], bf16, name="xt_sb")
            x_sb = pers.tile([128, NB, DF], bf16, name="x_sb")
            w_sb = pers.tile([128, HEADS, D_IN], bf16, name="w_sb")
            bias_sb = pers.tile([128, D_IN], f32, name="bias_sb")

            xt_r = xt.ap().rearrange("(h p) s -> p h s", p=128)
            x_r = x.ap().rearrange("(o p) f -> p o f", p=128)
            w_r = w.ap().rearrange("(h p) o -> p h o", p=128)

            # Front-load head 0 and the first-needed x blocks.
            nc.sync.dma_start(xt_sb[:, 0, 0:QH], xt_r[:, 0, 0:QH])
            nc.sync.dma_start(xt_sb[:, 0, QH:S], xt_r[:, 0, QH:S])
            for i in ORDER:
                nc.sync.dma_start(x_sb[:, i, :], x_r[:, i, :])
            for h in range(1, HEADS):
                nc.sync.dma_start(xt_sb[:, h, :], xt_r[:, h, :])
            for h in range(HEADS):
                nc.sync.dma_start(w_sb[:, h, :], w_r[:, h, :])
            nc.sync.dma_start(bias_sb[:], bias.ap())

            # fc accumulators, persist across heads
            accs = []
            for j in range(QB):
                a = pers.tile([128, D_IN], f32, name=f"acc{j}")
                accs.append(a)

            # previous head's deferred steps, popped one per slot
            pending = []

            def emit_step():
                if pending:
                    pending.pop(0)()

            state = {}

            def emit_fc_head(h, outT):
                for j in range(QB):
                    def step(h=h, j=j, outT=outT):
                        recip = state[h]
                        pfc = psFC.tile([128, D_IN], f32, tag="pfc", name="pfc")
                        nc.tensor.matmul(pfc[:], outT[:, ts(j, 128)],
                                         w_sb[:, h, :], start=True, stop=True)
                        if h == 0:
                            nc.vector.scalar_tensor_tensor(
                                accs[j][:], pfc[:], recip[:, j : j + 1],
                                bias_sb[:], mult, add)
                        else:
                            nc.vector.scalar_tensor_tensor(
                                accs[j][:], pfc[:], recip[:, j : j + 1],
                                accs[j][:], mult, add)
                        if h == HEADS - 1:
                            nc.sync.dma_start(out.ap()[ts(j, 128), :], accs[j][:])
                    pending.append(step)

            for h in range(HEADS):
                racc = pers.tile([128, HB], f32, name=f"racc{h}")
                rb_d = dram.tile([1, QH], f32, tag="rbd", name=f"rbd{h}", bufs=2)
                TsumB = tsb.tile([128, QH], bf16, tag="tsb", name="TsumB")

                pso = psO.tile([128, QH], f32, tag="pso", name="pso")
                pse_tiles = {}

                def mm1(i, h=h, pse_tiles=pse_tiles):
                    pse = psE.tile([128, QH], f32, tag="pse", name="pse")
                    lhs1 = xt_sb[:, h, ts(i, 128)]
                    nc.tensor.matmul(pse[:, 0:512], lhs1, xt_sb[:, h, 0:512],
                                     start=True, stop=True)
                    nc.tensor.matmul(pse[:, 512:1024], lhs1,
                                     xt_sb[:, h, 512:1024],
                                     start=True, stop=True)
                    pse_tiles[i] = pse

                def fin1(TsumB=TsumB, rb_d=rb_d):
                    rbb = rpool.tile([128, QH], f32, tag="rbb", name="rbb")
                    nc.gpsimd.partition_all_reduce(rbb[:], TsumB[:], 128, radd)
                    nc.sync.dma_start(rb_d[:], rbb[0:1, :])

                def fin2a(h=h, rb_d=rb_d):
                    rbt = rpool.tile([128, QB], f32, tag="rbt", name="rbt")
                    with nc.allow_non_contiguous_dma(reason="4KB rowsum load"):
                        nc.sync.dma_start(
                            rbt[:], rb_d[0, :].rearrange("(j p) -> p j", p=128))
                    state[(h, "rbt")] = rbt

                def fin2b(h=h, racc=racc):
                    rbt = state.pop((h, "rbt"))
                    rsum = rpool.tile([128, QB], f32, tag="rsum", name="rsum")
                    nc.vector.tensor_tensor(rsum[:], racc[:], rbt[:], add)
                    recip = rpool.tile([128, QB], f32, tag="recip", name="recip")
                    nc.vector.reciprocal(recip[:], rsum[:])
                    state[h] = recip

                mm1(ORDER[0])
                mm1(ORDER[1])
                for slot in range(NB):
                    i = ORDER[slot]
                    pse = pse_tiles.pop(i)
                    s_i = spool.tile([128, QH], bf16, tag="s", name="s_i")
                    if i < HB:
                        # symmetric-square tile: free-dim sums == my-half
                        # row-sums, in [q-part, block] layout
                        nc.scalar.activation(s_i[:], pse[:], Exp, bias=0.0,
                                             scale=ISQ,
                                             accum_out=racc[:, i : i + 1])
                    else:
                        nc.scalar.activation(s_i[:], pse[:], Exp, bias=0.0,
                                             scale=ISQ)
                    if slot + 2 < NB:
                        mm1(ORDER[slot + 2])
                    lhs2 = x_sb[:, i, ts(h, 128)]
                    nc.tensor.matmul(pso[:, 0:512], lhs2, s_i[:, 0:512],
                                     start=(slot == 0), stop=(slot == NB - 1))
                    nc.tensor.matmul(pso[:, 512:1024], lhs2, s_i[:, 512:1024],
                                     start=(slot == 0), stop=(slot == NB - 1))
                    if i >= HB:
                        # B-tile: accumulate for the partition-dim reduction
                        if slot == 0:
                            nc.vector.tensor_copy(TsumB[:], s_i[:])
                        else:
                            nc.vector.tensor_tensor(TsumB[:], TsumB[:], s_i[:],
                                                    add)
                    if slot == 8:
                        fin1()
                    elif slot == 10:
                        fin2a()
                    # previous head's deferred steps, one per slot in the
                    # second half of the loop (the first half carries the
                    # TsumB adds; keeps DVE evenly loaded)
                    if slot >= 6 and slot < 15:
                        emit_step()

                outT = outp.tile([128, QH], bf16, tag="outT", name="outT")
                nc.vector.tensor_copy(outT[:], pso[:])

                pending.append(fin2b)
                emit_fc_head(h, outT)

            while pending:
                emit_step()

    nc.compile()
    return nc


def _get_nc():
    global _CACHED_NC
    if _CACHED_NC is None:
        _CACHED_NC = _build_nc()
    return _CACHED_NC


def make_in_maps(output1, output2, W_out, b_out):
    bf = ml_dtypes.bfloat16
    X = np.concatenate([np.asarray(output1), np.asarray(output2)], axis=2)  # [N,S,DF]
    Xb = X.astype(bf)
    Wb = np.ascontiguousarray(np.asarray(W_out).astype(bf))
    bias_full = np.ascontiguousarray(
        np.broadcast_to(np.asarray(b_out).astype(np.float32), (128, D_IN)))

    in_maps = []
    for c in range(8):
        n, half = c // 2, c % 2
        Xn = Xb[n]                                   # [S, DF]
        # permute l so this core's own q-range rows come first
        if half == 0:
            Xp = np.ascontiguousarray(Xn)
        else:
            Xp = np.ascontiguousarray(
                np.concatenate([Xn[QH:], Xn[:QH]], axis=0))
        XTp = np.ascontiguousarray(Xp.T)             # [DF, S]
        in_maps.append({
            "x": Xp,
            "xt": XTp,
            "w": Wb,
            "bias": bias_full,
        })
    return in_maps


def kernel(output1, output2, W_out, b_out):
    from concourse.bass_utils import run_bass_kernel_spmd

    in_maps = make_in_maps(output1, output2, W_out, b_out)
    nc = _get_nc()
    res = run_bass_kernel_spmd(nc, in_maps, core_ids=list(range(8)))

    full = np.empty((N, S, D_IN), np.float32)
    for c in range(8):
        n, half = c // 2, c % 2
        full[n, half * QH : (half + 1) * QH, :] = res.results[c]["out"]
    return full
